# revision 7
# baseline (speedup 1.0000x reference)
"""Graves-style gaussian attention window (no offset) on 8 TRN2 cores, v8.

v3 + HAM warm-up and engine-balance tuning:
  - ~10 dummy fp16 matmuls on the const tile at kernel start keep the
    PE busy through the 3.4us HAM window so real matmuls run at 2.4GHz
  - single merged 4-bank exp ACT (bias folded into qsb / kill-bias)
  - esum tree (vec+gpsimd) -> only 4 out matmuls per block
  - lt loads as 2 DMAs per batch; small consts dispatched from VectorE
"""

import numpy as np

import concourse.bass as bass
import concourse.bacc as bacc
import concourse.tile as tile
from concourse import mybir
from concourse.bass_utils import run_bass_kernel_spmd

B, T, H = 16, 1024, 512
KG = 10
UC = 42
A = 80
U_IN = 600
NCORES = 8
BPC = B // NCORES
P = 128
TC = 512
NTC = T // TC
HC = H // P
NBLK = BPC * NTC
NWARM = 20
FP32 = mybir.dt.float32
FP16 = mybir.dt.float16
LN2 = float(np.log(np.float32(2.0)))
EXP = mybir.ActivationFunctionType.Exp

CF_U8Q = 0
CF_U8PA = 128
CF_W2 = 256
CF_CHS = 768

_cache: dict = {}


def _build_program() -> bass.Bass:
    nc = bacc.Bacc("TRN2", target_bir_lowering=False, debug=False)
    lstmT = nc.declare_dram_parameter("lstmT", [BPC, 2, H // 2, T], FP16,
                                      isOutput=False)
    CF = nc.declare_dram_parameter("CF", [P, 928], FP16, isOutput=False)
    BF = nc.declare_dram_parameter("BF", [P, 2], FP32, isOutput=False)
    outD = nc.declare_dram_parameter("outD", [BPC, P, NTC, TC // P, A],
                                     FP32, isOutput=True)

    with tile.TileContext(nc) as tc, \
            nc.allow_low_precision("fp16 pipeline by design"), \
            tc.tile_pool(name="consts", bufs=1) as consts, \
            tc.tile_pool(name="ltp", bufs=8) as ltp, \
            tc.tile_pool(name="dp", bufs=2) as dp, \
            tc.tile_pool(name="sp", bufs=6) as sp, \
            tc.tile_pool(name="e4p", bufs=2) as e4p, \
            tc.tile_pool(name="osp", bufs=2) as osp, \
            tc.tile_pool(name="qps", bufs=1, space="PSUM") as qps, \
            tc.tile_pool(name="eps", bufs=3, space="PSUM") as eps, \
            tc.tile_pool(name="ops", bufs=1, space="PSUM") as ops:

        cf = consts.tile([P, 928], FP16, name="cf")
        nc.sync.dma_start(out=cf, in_=CF[:, :])
        # HAM warm-up: the PE runs at 1.2GHz until it has been busy
        # through one free-running 3.4us activity window, so burn that
        # window on cheap N=128 matmuls while the input DMAs are in
        # flight; memset source so no DMA completion gates the first MM
        wt = sp.tile([P, TC], FP16, name="wt", tag="wt")
        nc.gpsimd.memset(wt, 0.0)
        qw = qps.tile([P, TC], FP32, name="qw", tag="q")
        for w in range(NWARM):
            nc.tensor.matmul(out=qw, lhsT=wt[:, 0:P], rhs=wt,
                             start=(w == 0), stop=(w == NWARM - 1))
        lts = {}
        dma_engines = [nc.sync, nc.scalar, nc.gpsimd, nc.scalar,
                       nc.sync, nc.scalar, nc.gpsimd, nc.sync]
        for b in range(BPC):
            for h in range(2):
                for hh in range(2):
                    c = 2 * h + hh
                    lt_ = ltp.tile([P, T], FP16, name=f"lt_{b}_{c}",
                                   tag="lt")
                    eng = dma_engines[b * 4 + c]
                    eng.dma_start(
                        out=lt_,
                        in_=lstmT[b, h, hh * P:(hh + 1) * P, :])
                    lts[(b, c)] = lt_

        bf = consts.tile([P, 2], FP32, name="bf")
        nc.gpsimd.dma_start(out=bf, in_=BF[:, :])

        st: dict = {}
        osbs: dict = {}

        def P1(k):
            b, tci = divmod(k, NTC)
            tsl = slice(tci * TC, (tci + 1) * TC)
            q = qps.tile([P, TC], FP32, name=f"q_{k}", tag="q")
            for c in range(HC):
                nc.tensor.matmul(
                    out=q, lhsT=cf[:, CF_W2 + c * P:CF_W2 + (c + 1) * P],
                    rhs=lts[(b, c)][:, tsl],
                    start=(c == 0), stop=(c == HC - 1))
            D = dp.tile([P, TC], FP32, name=f"D_{k}", tag="D")
            nc.scalar.activation(out=D, in_=q, func=EXP,
                                 bias=bf[:, 0:1], scale=1.0)
            Dh = sp.tile([P, TC], FP16, name=f"Dh_{k}", tag="Dh")
            nc.vector.tensor_copy(out=Dh, in_=D)
            Dl = sp.tile([P, TC], FP16, name=f"Dl_{k}", tag="Dl")
            nc.vector.tensor_sub(out=Dl, in0=D, in1=Dh)
            qsb = sp.tile([P, TC], FP16, name=f"qsb_{k}", tag="qsb")
            nc.vector.tensor_scalar_add(out=qsb, in0=q, scalar1=bf[:, 1:2])
            st[k] = {"Dh": Dh, "Dl": Dl, "qsb": qsb}

        def P2(k):
            if k == 0:
                # block 0 has no neighbor work to pipeline: keep the PE
                # busy through the cast/sub window so HAM stays at 8/8
                fw = ops.tile([P, TC // P, A], FP32, name="fw", tag="op")
                for w in range(10):
                    nc.tensor.matmul(out=fw, lhsT=wt[:, 0:P],
                                     rhs=wt[:, 0:(TC // P) * A],
                                     start=(w == 0), stop=(w == 9))
            Dh, Dl, qsb = st[k]["Dh"], st[k]["Dl"], st[k]["qsb"]
            epA = eps.tile([P, 2, TC], FP32, name=f"epA_{k}", tag="ep")
            epB = eps.tile([P, 2, TC], FP32, name=f"epB_{k}", tag="ep")
            for i in range(4):
                r = 32 * i
                ep = epA if i < 2 else epB
                nc.tensor.matmul(
                    out=ep[:, i % 2, :],
                    lhsT=cf[r:r + 12, CF_U8Q:CF_U8Q + P],
                    rhs=Dh[r:r + 12, :], start=True, stop=False,
                    tile_position=(r, 0))
            for i in range(4):
                r = 32 * i
                ep = epA if i < 2 else epB
                nc.tensor.matmul(
                    out=ep[:, i % 2, :],
                    lhsT=cf[r:r + 12, CF_U8Q:CF_U8Q + P],
                    rhs=Dl[r:r + 12, :], start=False, stop=False,
                    tile_position=(r, 0))
            for i in range(4):
                r = 32 * i
                ep = epA if i < 2 else epB
                nc.tensor.matmul(
                    out=ep[:, i % 2, :],
                    lhsT=cf[r:r + 12, CF_U8PA:CF_U8PA + P],
                    rhs=qsb[r:r + 12, :], start=False, stop=True,
                    tile_position=(r, 0))
            e4 = e4p.tile([P, 4, TC], FP16, name=f"e4_{k}", tag="e4")
            nc.scalar.activation(out=e4[:, 0:2, :], in_=epA, func=EXP)
            nc.scalar.activation(out=e4[:, 2:4, :], in_=epB, func=EXP)
            st[k]["e4"] = e4

        def P3(k):
            b, tci = divmod(k, NTC)
            e4 = st[k]["e4"]
            s01 = sp.tile([P, TC], FP16, name=f"s01_{k}", tag="s")
            nc.vector.tensor_add(out=s01, in0=e4[:, 0, :], in1=e4[:, 1, :])
            s23 = sp.tile([P, TC], FP16, name=f"s23_{k}", tag="s")
            nc.gpsimd.tensor_add(out=s23, in0=e4[:, 2, :], in1=e4[:, 3, :])
            op = ops.tile([P, TC // P, A], FP32, name=f"op_{k}", tag="op")
            chs = cf[:, CF_CHS + A * b:CF_CHS + A * (b + 1)]
            for ts in range(TC // P):
                nc.tensor.matmul(
                    out=op[:, ts, :], lhsT=s01[:, ts * P:(ts + 1) * P],
                    rhs=chs, start=True, stop=False)
                nc.tensor.matmul(
                    out=op[:, ts, :], lhsT=s23[:, ts * P:(ts + 1) * P],
                    rhs=chs, start=False, stop=True)
            osb = osp.tile([P, TC // P, A], FP32, name=f"osb_{k}", tag="osb")
            nc.vector.tensor_copy(out=osb, in_=op)
            nc.sync.dma_start(out=outD[b, :, tci, :, :], in_=osb)
            del st[k]

        for k in range(NBLK + 2):
            if k < NBLK:
                P1(k)
            if 1 <= k <= NBLK:
                P2(k - 1)
            if k >= 2:
                P3(k - 2)
    nc.compile()
    return nc


def _host_prep(lstm_out, char_seq, W, bias):
    lstm_out = np.asarray(lstm_out, dtype=np.float32)
    char_seq = np.asarray(char_seq, dtype=np.float32)
    W = np.asarray(W, dtype=np.float32)
    bias = np.asarray(bias, dtype=np.float32)

    W2 = np.zeros((H, P), np.float32)
    BQ = np.zeros((P, 2), np.float32)
    for g in range(KG):
        i, j = g // 3, g % 3
        r = 32 * i + 3 * j
        W2[:, r + 0] = W[:, 10 + g]
        W2[:, r + 1] = W[:, 10 + g] + W[:, 20 + g]
        W2[:, r + 2] = W[:, 10 + g] + 2.0 * W[:, 20 + g]
        BQ[r + 0, 0] = bias[10 + g]
        BQ[r + 1, 0] = bias[10 + g] + bias[20 + g] + LN2
        BQ[r + 2, 0] = bias[10 + g] + 2.0 * bias[20 + g]
        W2[:, 32 * i + 9 + j] = W[:, g]
        BQ[32 * i + 9 + j, 1] = bias[g]
    for j in (1, 2):
        BQ[32 * 3 + 3 * j + 2, 0] = 10.0

    u = np.arange(UC, dtype=np.float32)
    u8q = np.zeros((12, P), np.float32)
    u8pa = np.zeros((12, P), np.float32)
    for j in range(3):
        u8q[3 * j + 0, UC * j:UC * (j + 1)] = -u * u
        u8q[3 * j + 1, UC * j:UC * (j + 1)] = u
        u8q[3 * j + 2, UC * j:UC * (j + 1)] = -1.0
        u8pa[9 + j, UC * j:UC * (j + 1)] = 1.0
    u8q_rep = np.zeros((P, P), np.float32)
    u8pa_rep = np.zeros((P, P), np.float32)
    for i in range(4):
        u8q_rep[32 * i:32 * i + 12] = u8q
        u8pa_rep[32 * i:32 * i + 12] = u8pa

    w2s = np.ascontiguousarray(
        W2.reshape(HC, P, P).transpose(1, 0, 2).reshape(P, HC * P))

    ch = char_seq.reshape(NCORES, BPC, U_IN, A)[:, :, :UC, :]
    char3 = np.zeros((NCORES, BPC, P, A), np.float32)
    for j in range(3):
        char3[:, :, UC * j:UC * (j + 1), :] = ch
    chs = char3.transpose(0, 2, 1, 3).reshape(NCORES, P, BPC * A)

    lstmT = lstm_out.reshape(NCORES, BPC, T, H).transpose(0, 1, 3, 2)
    lstmT = np.ascontiguousarray(lstmT).astype(np.float16)
    lstmT = lstmT.reshape(NCORES, BPC, 2, H // 2, T)

    base = np.concatenate([u8q_rep, u8pa_rep, w2s], axis=1)
    in_maps = []
    for i in range(NCORES):
        cfi = np.concatenate([base, chs[i]], axis=1).astype(np.float16)
        in_maps.append({
            "lstmT": lstmT[i],
            "CF": np.ascontiguousarray(cfi),
            "BF": BQ,
        })
    return in_maps


def kernel(lstm_out, char_seq, W, bias, _trace=False, _tmpdir=None):
    if "nc" not in _cache:
        _cache["nc"] = _build_program()
    nc = _cache["nc"]
    in_maps = _host_prep(lstm_out, char_seq, W, bias)
    res = run_bass_kernel_spmd(nc, in_maps, list(range(NCORES)),
                               trace=_trace, tmpdir=_tmpdir)
    if _trace:
        _cache["last"] = res
    outs = []
    for i in range(NCORES):
        o = res.results[i]["outD"]
        o = o.transpose(0, 2, 3, 1, 4).reshape(BPC, T, A)
        outs.append(o)
    return np.ascontiguousarray(
        np.concatenate(outs, axis=0), dtype=np.float32)


# revision 8
# speedup vs baseline: 1.2353x; 1.2353x over previous
"""Graves-style gaussian attention window (no offset) on 8 TRN2 cores, v8.

v3 + HAM warm-up and engine-balance tuning:
  - ~10 dummy fp16 matmuls on the const tile at kernel start keep the
    PE busy through the 3.4us HAM window so real matmuls run at 2.4GHz
  - single merged 4-bank exp ACT (bias folded into qsb / kill-bias)
  - esum tree (vec+gpsimd) -> only 4 out matmuls per block
  - lt loads as 2 DMAs per batch; small consts dispatched from VectorE
"""

import numpy as np

import concourse.bass as bass
import concourse.bacc as bacc
import concourse.tile as tile
from concourse import mybir
from concourse.bass_utils import run_bass_kernel_spmd

B, T, H = 16, 1024, 512
KG = 10
UC = 42
A = 80
U_IN = 600
NCORES = 8
BPC = B // NCORES
P = 128
TC = 512
NTC = T // TC
HC = H // P
NBLK = BPC * NTC
NWARM = 20
FP32 = mybir.dt.float32
FP16 = mybir.dt.float16
LN2 = float(np.log(np.float32(2.0)))
EXP = mybir.ActivationFunctionType.Exp

CF_U8Q = 0
CF_U8PA = 128
CF_W2 = 256
CF_CHS = 768

_cache: dict = {}


def _build_program() -> bass.Bass:
    nc = bacc.Bacc("TRN2", target_bir_lowering=False, debug=False)
    lstmT = nc.declare_dram_parameter("lstmT", [BPC, 2, H // 2, T], FP16,
                                      isOutput=False)
    CF = nc.declare_dram_parameter("CF", [P, 928], FP16, isOutput=False)
    BF = nc.declare_dram_parameter("BF", [P, 2], FP32, isOutput=False)
    outD = nc.declare_dram_parameter("outD", [BPC, P, NTC, TC // P, A],
                                     FP32, isOutput=True)

    with tile.TileContext(nc) as tc, \
            nc.allow_low_precision("fp16 pipeline by design"), \
            tc.tile_pool(name="consts", bufs=1) as consts, \
            tc.tile_pool(name="ltp", bufs=8) as ltp, \
            tc.tile_pool(name="dp", bufs=2) as dp, \
            tc.tile_pool(name="sp", bufs=6) as sp, \
            tc.tile_pool(name="e4p", bufs=2) as e4p, \
            tc.tile_pool(name="osp", bufs=2) as osp, \
            tc.tile_pool(name="qps", bufs=1, space="PSUM") as qps, \
            tc.tile_pool(name="eps", bufs=3, space="PSUM") as eps, \
            tc.tile_pool(name="ops", bufs=1, space="PSUM") as ops:

        cf = consts.tile([P, 928], FP16, name="cf")
        nc.sync.dma_start(out=cf, in_=CF[:, :])
        # HAM warm-up: the PE runs at 1.2GHz until it has been busy
        # through one free-running 3.4us activity window, so burn that
        # window on cheap N=128 matmuls while the input DMAs are in
        # flight; memset source so no DMA completion gates the first MM
        wt = sp.tile([P, TC], FP16, name="wt", tag="wt")
        nc.gpsimd.memset(wt, 0.0)
        qw = qps.tile([P, TC], FP32, name="qw", tag="q")
        for w in range(NWARM):
            nc.tensor.matmul(out=qw, lhsT=wt[:, 0:P], rhs=wt,
                             start=(w == 0), stop=(w == NWARM - 1))
        lts = {}
        dma_engines = [nc.sync, nc.scalar, nc.gpsimd, nc.scalar,
                       nc.sync, nc.scalar, nc.gpsimd, nc.sync]
        for b in range(BPC):
            for h in range(2):
                for hh in range(2):
                    c = 2 * h + hh
                    lt_ = ltp.tile([P, T], FP16, name=f"lt_{b}_{c}",
                                   tag="lt")
                    eng = dma_engines[b * 4 + c]
                    eng.dma_start(
                        out=lt_,
                        in_=lstmT[b, h, hh * P:(hh + 1) * P, :])
                    lts[(b, c)] = lt_

        bf = consts.tile([P, 2], FP32, name="bf")
        nc.gpsimd.dma_start(out=bf, in_=BF[:, :])

        st: dict = {}
        osbs: dict = {}

        def P1(k):
            b, tci = divmod(k, NTC)
            tsl = slice(tci * TC, (tci + 1) * TC)
            q = qps.tile([P, TC], FP32, name=f"q_{k}", tag="q")
            for c in range(HC):
                nc.tensor.matmul(
                    out=q, lhsT=cf[:, CF_W2 + c * P:CF_W2 + (c + 1) * P],
                    rhs=lts[(b, c)][:, tsl],
                    start=(c == 0), stop=(c == HC - 1))
            D = dp.tile([P, TC], FP32, name=f"D_{k}", tag="D")
            nc.scalar.activation(out=D, in_=q, func=EXP,
                                 bias=bf[:, 0:1], scale=1.0)
            qsb = sp.tile([P, TC], FP16, name=f"qsb_{k}", tag="qsb")
            nc.vector.tensor_scalar_add(out=qsb, in0=q, scalar1=bf[:, 1:2])
            Dh = sp.tile([P, TC], FP16, name=f"Dh_{k}", tag="Dh")
            nc.vector.tensor_copy(out=Dh, in_=D)
            Dl = sp.tile([P, TC], FP16, name=f"Dl_{k}", tag="Dl")
            nc.vector.tensor_sub(out=Dl, in0=D, in1=Dh)
            st[k] = {"Dh": Dh, "Dl": Dl, "qsb": qsb}

        def P2(k):
            if k == 0:
                # block 0 has no neighbor work to pipeline: keep the PE
                # busy through the cast/sub window so HAM stays at 8/8
                fw = ops.tile([P, TC // P, A], FP32, name="fw", tag="op")
                for w in range(10):
                    nc.tensor.matmul(out=fw, lhsT=wt[:, 0:P],
                                     rhs=wt[:, 0:(TC // P) * A],
                                     start=(w == 0), stop=(w == 9))
            Dh, Dl, qsb = st[k]["Dh"], st[k]["Dl"], st[k]["qsb"]
            epA = eps.tile([P, 2, TC], FP32, name=f"epA_{k}", tag="ep")
            epB = eps.tile([P, 2, TC], FP32, name=f"epB_{k}", tag="ep")
            for i in range(4):
                r = 32 * i
                ep = epA if i < 2 else epB
                nc.tensor.matmul(
                    out=ep[:, i % 2, :],
                    lhsT=cf[r:r + 12, CF_U8Q:CF_U8Q + P],
                    rhs=Dh[r:r + 12, :], start=True, stop=False,
                    tile_position=(r, 0))
            for i in range(4):
                r = 32 * i
                ep = epA if i < 2 else epB
                nc.tensor.matmul(
                    out=ep[:, i % 2, :],
                    lhsT=cf[r:r + 12, CF_U8Q:CF_U8Q + P],
                    rhs=Dl[r:r + 12, :], start=False, stop=False,
                    tile_position=(r, 0))
            for i in range(4):
                r = 32 * i
                ep = epA if i < 2 else epB
                nc.tensor.matmul(
                    out=ep[:, i % 2, :],
                    lhsT=cf[r:r + 12, CF_U8PA:CF_U8PA + P],
                    rhs=qsb[r:r + 12, :], start=False, stop=True,
                    tile_position=(r, 0))
            e4 = e4p.tile([P, 4, TC], FP16, name=f"e4_{k}", tag="e4")
            nc.scalar.activation(out=e4[:, 0:2, :], in_=epA, func=EXP)
            nc.scalar.activation(out=e4[:, 2:4, :], in_=epB, func=EXP)
            st[k]["e4"] = e4

        def P3(k):
            b, tci = divmod(k, NTC)
            e4 = st[k]["e4"]
            s01 = sp.tile([P, TC], FP16, name=f"s01_{k}", tag="s")
            nc.vector.tensor_add(out=s01, in0=e4[:, 0, :], in1=e4[:, 1, :])
            s23 = sp.tile([P, TC], FP16, name=f"s23_{k}", tag="s")
            nc.gpsimd.tensor_add(out=s23, in0=e4[:, 2, :], in1=e4[:, 3, :])
            op = ops.tile([P, TC // P, A], FP32, name=f"op_{k}", tag="op")
            chs = cf[:, CF_CHS + A * b:CF_CHS + A * (b + 1)]
            for ts in range(TC // P):
                nc.tensor.matmul(
                    out=op[:, ts, :], lhsT=s01[:, ts * P:(ts + 1) * P],
                    rhs=chs, start=True, stop=False)
                nc.tensor.matmul(
                    out=op[:, ts, :], lhsT=s23[:, ts * P:(ts + 1) * P],
                    rhs=chs, start=False, stop=True)
            osb = osp.tile([P, TC // P, A], FP32, name=f"osb_{k}", tag="osb")
            nc.vector.tensor_copy(out=osb, in_=op)
            nc.sync.dma_start(out=outD[b, :, tci, :, :], in_=osb)
            del st[k]

        for k in range(NBLK + 2):
            if k < NBLK:
                P1(k)
            if 1 <= k <= NBLK:
                P2(k - 1)
            if k >= 2:
                P3(k - 2)
    nc.compile()
    return nc


def _host_prep(lstm_out, char_seq, W, bias):
    lstm_out = np.asarray(lstm_out, dtype=np.float32)
    char_seq = np.asarray(char_seq, dtype=np.float32)
    W = np.asarray(W, dtype=np.float32)
    bias = np.asarray(bias, dtype=np.float32)

    W2 = np.zeros((H, P), np.float32)
    BQ = np.zeros((P, 2), np.float32)
    for g in range(KG):
        i, j = g // 3, g % 3
        r = 32 * i + 3 * j
        W2[:, r + 0] = W[:, 10 + g]
        W2[:, r + 1] = W[:, 10 + g] + W[:, 20 + g]
        W2[:, r + 2] = W[:, 10 + g] + 2.0 * W[:, 20 + g]
        BQ[r + 0, 0] = bias[10 + g]
        BQ[r + 1, 0] = bias[10 + g] + bias[20 + g] + LN2
        BQ[r + 2, 0] = bias[10 + g] + 2.0 * bias[20 + g]
        W2[:, 32 * i + 9 + j] = W[:, g]
        BQ[32 * i + 9 + j, 1] = bias[g]
    for j in (1, 2):
        BQ[32 * 3 + 3 * j + 2, 0] = 10.0

    u = np.arange(UC, dtype=np.float32)
    u8q = np.zeros((12, P), np.float32)
    u8pa = np.zeros((12, P), np.float32)
    for j in range(3):
        u8q[3 * j + 0, UC * j:UC * (j + 1)] = -u * u
        u8q[3 * j + 1, UC * j:UC * (j + 1)] = u
        u8q[3 * j + 2, UC * j:UC * (j + 1)] = -1.0
        u8pa[9 + j, UC * j:UC * (j + 1)] = 1.0
    u8q_rep = np.zeros((P, P), np.float32)
    u8pa_rep = np.zeros((P, P), np.float32)
    for i in range(4):
        u8q_rep[32 * i:32 * i + 12] = u8q
        u8pa_rep[32 * i:32 * i + 12] = u8pa

    w2s = np.ascontiguousarray(
        W2.reshape(HC, P, P).transpose(1, 0, 2).reshape(P, HC * P))

    ch = char_seq.reshape(NCORES, BPC, U_IN, A)[:, :, :UC, :]
    char3 = np.zeros((NCORES, BPC, P, A), np.float32)
    for j in range(3):
        char3[:, :, UC * j:UC * (j + 1), :] = ch
    chs = char3.transpose(0, 2, 1, 3).reshape(NCORES, P, BPC * A)

    lstmT = lstm_out.reshape(NCORES, BPC, T, H).transpose(0, 1, 3, 2)
    lstmT = np.ascontiguousarray(lstmT).astype(np.float16)
    lstmT = lstmT.reshape(NCORES, BPC, 2, H // 2, T)

    base = np.concatenate([u8q_rep, u8pa_rep, w2s], axis=1)
    in_maps = []
    for i in range(NCORES):
        cfi = np.concatenate([base, chs[i]], axis=1).astype(np.float16)
        in_maps.append({
            "lstmT": lstmT[i],
            "CF": np.ascontiguousarray(cfi),
            "BF": BQ,
        })
    return in_maps


def kernel(lstm_out, char_seq, W, bias, _trace=False, _tmpdir=None):
    if "nc" not in _cache:
        _cache["nc"] = _build_program()
    nc = _cache["nc"]
    in_maps = _host_prep(lstm_out, char_seq, W, bias)
    res = run_bass_kernel_spmd(nc, in_maps, list(range(NCORES)),
                               trace=_trace, tmpdir=_tmpdir)
    if _trace:
        _cache["last"] = res
    outs = []
    for i in range(NCORES):
        o = res.results[i]["outD"]
        o = o.transpose(0, 2, 3, 1, 4).reshape(BPC, T, A)
        outs.append(o)
    return np.ascontiguousarray(
        np.concatenate(outs, axis=0), dtype=np.float32)
